# revision 9
# baseline (speedup 1.0000x reference)
"""2-layer GCN (GCNConv x2 + log_softmax) on 8 trn2 NeuronCores via Bass/Tile.

Math (identical to the reference by associativity + rank-1 factorization of
the symmetric normalization):
  dis = rsqrt(deg) with self-loops;  A_hat = D^-1/2 (A+I) D^-1/2
  L1: h1 = relu(dis * (segsum(T1[src]) + T1[own]) + b1),  T1 = dis * (x @ W1)
  L2: y  = log_softmax((dis * (segsum(T2[src]) + T2[own])) @ W2 + b2),
      T2 = dis * h1
(the self-loop term is T[own] since norm = dis^2 there; W2 commutes with the
 aggregation by linearity, so both edge passes move 16-wide rows).

Device strategy (per core, dst-sharded nodes):
  - node tables stored as 256B tokens (16 valid f32 + 48 never-read) so the
    Pool-engine bulk `dma_gather` (InstDMAGatherAnt, mlp gpsimd library) can
    fetch one token per edge: ~10ns/edge vs ~50ns/edge for per-[P,1]
    indirect DMAs.
  - int16 gather indices only address 32767 tokens, so each tile's edge list
    is split into 4 source-quadrant streams (25000 tokens each).
  - scatter+segment-reduce is done on the TensorEngine: per 128-edge window
    build a one-hot D[e, dst] = (iota == dstloc_e) on the DVE and matmul
    D^T @ gathered[:, :16] into the dst tile's PSUM accumulator. Padded
    edges get dstloc=200 -> zero column -> no contribution.
  - windows are emitted per (8-tile block, quadrant) so gather calls pack up
    to 8 windows (1024 idxs; the SWDGE ring holds 128 entries = 2032 idxs
    max per call, and >1024 was observed to wedge the device).
"""

import numpy as np

import concourse.bass as bass
import concourse.mybir as mybir
import concourse.tile as tile
from concourse import library_config
from concourse.library_overlay import lower_extended_insts
from concourse.masks import make_identity
from concourse.vector_clock import ScopedClock

P = 128
F1 = 16
F2 = 40
D = 512
N_NODES = 100000
N_CORES = 8
S = N_NODES // N_CORES          # 12500
T = (S + P - 1) // P            # 98 tiles (97*128 + 84)
NQ = 4
QUAD = N_NODES // NQ            # 25000 tokens per int16-addressable window
TOK = 64                        # f32 per 256B token
WCALL = 8                       # max windows (1024 idxs) per dma_gather
BLOCK = 8                       # dst tiles per scheduling block
SENT = 200.0                    # dstloc sentinel for padded edges

# ---------------------------------------------------------------------------
# workaround: this walrus build rejects >1 sync wait per instruction and the
# Drain opcode; spill extra waits onto single-wait nops.
_nop_counter = [0]


def _fresh_nop(engine, wait):
    _nop_counter[0] += 1
    nop = mybir.InstNoOp(name=f"WSPILL-{_nop_counter[0]}", ins=[], outs=[])
    nop.engine = engine
    nop.sync_info = mybir.SyncInfo(on_wait=[wait], on_update=[])
    return nop


def _split_multi_waits(nc):
    for fn in nc.m.functions:
        for bb in fn.blocks:
            insts = bb.instructions
            if not any(
                i.sync_info is not None and len(i.sync_info.on_wait) > 1
                for i in insts
            ):
                continue
            newlist = []
            for inst in insts:
                si = inst.sync_info
                if si is not None and len(si.on_wait) > 1:
                    waits = list(si.on_wait)
                    for w in waits[:-1]:
                        newlist.append(_fresh_nop(inst.engine, w))
                    si.on_wait = waits[-1:]
                    inst.sync_info = si
                newlist.append(inst)
            insts[:] = newlist


def _patched_drain_and_barrier(self, tick_clock, wait_clock):
    nc = self.nc
    drain_inst = nc.sync.nop(nofuse=True, hint="tail_drain_nop")
    wait_clock.add_sem_waits(
        drain_inst.ins, ScopedClock({None: tick_clock.global_clock})
    )
    nc.all_engine_barrier()
    assert self.sems is not None
    popped = nc._tile_sem_poison_stack.pop()
    assert popped is self._sem_poison
    nc.clear_and_free_semaphores(list(self.sems.allocated().values()))
    nc.all_engine_barrier()


tile.TileContext._drain_and_barrier = _patched_drain_and_barrier


# ---------------------------------------------------------------------------
def _preprocess(edge_index):
    """Shared (cross-core) window schedule + per-core gather streams."""
    e = np.asarray(edge_index)
    src = e[0].astype(np.int64)
    dst = e[1].astype(np.int64)
    deg = np.bincount(dst, minlength=N_NODES).astype(np.float32) + 1.0
    dis = (1.0 / np.sqrt(deg)).astype(np.float32)

    core = dst // S
    tl = (dst % S) // P
    quad = src // QUAD
    dloc = (dst % S) % P
    key = (core * T + tl) * NQ + quad
    order = np.argsort(key, kind="stable")
    ss = src[order]
    dl = dloc[order]
    bounds = np.searchsorted(key[order], np.arange(N_CORES * T * NQ + 1))
    cnt = np.diff(bounds).reshape(N_CORES, T, NQ)

    nwin = (cnt.max(axis=0) + P - 1) // P          # [T, NQ] shared
    assert (nwin.sum(axis=1) > 0).all()

    # schedule: tile-major (a PSUM bank allows one open accumulation group,
    # so each tile's chain must be contiguous); per (tile, quadrant) run,
    # chopped into balanced <=WCALL-window calls.
    # each call: (quad, [(tile, first, last), ...])
    wtot_tile = nwin.sum(axis=1)                   # windows per tile
    sched = []
    worder = []                                    # (t, q, w) in stream order
    emitted = np.zeros(T, np.int64)
    for t in range(T):
        for q in range(NQ):
            run = []
            for w in range(nwin[t, q]):
                first = emitted[t] == 0
                emitted[t] += 1
                last = emitted[t] == wtot_tile[t]
                run.append((t, first, last))
                worder.append((t, q, w))
            nc_ = -(-len(run) // WCALL)            # balanced call split
            for i in range(nc_):
                lo = i * len(run) // nc_
                hi = (i + 1) * len(run) // nc_
                sched.append((q, run[lo:hi]))
    wtot = len(worder)
    idxcols = sum(len(wins) * 8 for _, wins in sched)

    # per-core streams
    idx_arrs, dl_arrs = [], []
    for c in range(N_CORES):
        idxs = np.zeros((P, idxcols), np.int16)
        dlocs = np.full((P, wtot), SENT, np.float32)
        col = 0
        wof = 0
        for q, wins in sched:
            nw = len(wins)
            a = np.zeros(nw * P, np.int16)
            for j, (t, _, _) in enumerate(wins):
                # j-th window of this call == worder[wof + j]
                _, qq, w = worder[wof + j]
                k = (c * T + t) * NQ + qq
                lo = bounds[k] + w * P
                n = min(P, bounds[k + 1] - lo)
                if n > 0:
                    a[j * P:j * P + n] = (ss[lo:lo + n] - qq * QUAD).astype(
                        np.int16)
                    dlocs[:n, wof + j] = dl[lo:lo + n].astype(np.float32)
            idxs[:, col:col + nw * 8] = np.tile(
                a.reshape(-1, 16).T, (8, 1))
            col += nw * 8
            wof += nw
        idx_arrs.append(idxs)
        dl_arrs.append(dlocs)

    disq = np.ones((N_CORES, P, T), np.float32)
    for c in range(N_CORES):
        tmp = np.ones(T * P, np.float32)
        tmp[:S] = dis[c * S:(c + 1) * S]
        disq[c] = tmp.reshape(T, P).T  # [p, t] = dis[lo + t*P + p]
    meta = dict(sched=sched, idxcols=idxcols, wtot=wtot)
    percore = dict(idx=idx_arrs, dl=dl_arrs, disq=disq)
    return meta, percore


# ---------------------------------------------------------------------------
def _build_program(meta):
    sched, idxcols, wtot = meta["sched"], meta["idxcols"], meta["wtot"]
    fp = mybir.dt.float32

    nc = bass.Bass("TRN2", target_bir_lowering=False, debug=False,
                   num_devices=N_CORES, detect_race_conditions=False)
    x_in = nc.declare_dram_parameter("x", [S, D], fp, isOutput=False)
    w1_in = nc.declare_dram_parameter("W1", [D, F1], fp, isOutput=False)
    b1_in = nc.declare_dram_parameter("b1", [1, F1], fp, isOutput=False)
    w2_in = nc.declare_dram_parameter("W2", [F1, F2], fp, isOutput=False)
    b2_in = nc.declare_dram_parameter("b2", [1, F2], fp, isOutput=False)
    disq_in = nc.declare_dram_parameter("disq", [P, T], fp, isOutput=False)
    iota_in = nc.declare_dram_parameter("iotaM", [P, P], fp, isOutput=False)
    idx_in = nc.declare_dram_parameter("idx", [P, idxcols], mybir.dt.int16,
                                       isOutput=False)
    dl_in = nc.declare_dram_parameter("dl", [P, wtot], fp, isOutput=False)
    y_out = nc.declare_dram_parameter("y", [T * P, F2], fp, isOutput=True)

    q2 = nc.dram_tensor("q2", [S, TOK], fp)
    u2 = nc.dram_tensor("u2", [S, TOK], fp)
    TAB1 = nc.dram_tensor("TAB1", [N_NODES, TOK], fp, addr_space="Shared")
    TAB2 = nc.dram_tensor("TAB2", [N_NODES, TOK], fp, addr_space="Shared")
    groups = [list(range(N_CORES))]

    with tile.TileContext(nc) as tc:
        with tc.tile_pool(name="const", bufs=1) as cpool, \
             tc.tile_pool(name="xp", bufs=3) as xp, \
             tc.tile_pool(name="xtp", bufs=3) as xtp, \
             tc.tile_pool(name="pst", bufs=2, space="PSUM") as pst, \
             tc.tile_pool(name="hp", bufs=2, space="PSUM") as hp, \
             tc.tile_pool(name="agg", bufs=2, space="PSUM") as aggp, \
             tc.tile_pool(name="wps", bufs=2, space="PSUM") as wps, \
             tc.tile_pool(name="sl", bufs=3) as sl, \
             tc.tile_pool(name="dlp", bufs=3) as dlp, \
             tc.tile_pool(name="gp", bufs=4) as gp, \
             tc.tile_pool(name="dp", bufs=4) as dp, \
             tc.tile_pool(name="ep", bufs=8) as ep, \
             tc.tile_pool(name="ou", bufs=3) as ou:

            nc.gpsimd.load_library(library_config.mlp)
            nws = sorted({len(w) for _, w in sched})
            regs = {nw: nc.gpsimd.to_reg(nw * P) for nw in nws}

            ident = cpool.tile([P, P], fp)
            make_identity(nc, ident[:])
            w1s = cpool.tile([P, (D // P) * F1], fp)
            nc.sync.dma_start(
                w1s[:].rearrange("p (k f) -> p k f", f=F1),
                w1_in.ap().rearrange("(k p) f -> p k f", p=P),
            )
            w2s = cpool.tile([F1, F2], fp)
            nc.sync.dma_start(w2s[:], w2_in[:, :])
            ones_row = cpool.tile([1, P], fp)
            nc.vector.memset(ones_row[:], 1.0)
            b1row = cpool.tile([1, F1], fp)
            nc.sync.dma_start(b1row[:], b1_in[:, :])
            b2row = cpool.tile([1, F2], fp)
            nc.sync.dma_start(b2row[:], b2_in[:, :])
            b1ps = hp.tile([P, F1], fp, space="PSUM", tag="hp")
            nc.tensor.matmul(b1ps[:], lhsT=ones_row[:], rhs=b1row[:],
                             start=True, stop=True)
            b1t = cpool.tile([P, F1], fp)
            nc.vector.tensor_copy(b1t[:], b1ps[:])
            b2ps = wps.tile([P, F2], fp, space="PSUM", tag="wp")
            nc.tensor.matmul(b2ps[:], lhsT=ones_row[:], rhs=b2row[:],
                             start=True, stop=True)
            b2t = cpool.tile([P, F2], fp)
            nc.vector.tensor_copy(b2t[:], b2ps[:])
            disq = cpool.tile([P, T], fp)
            nc.sync.dma_start(disq[:], disq_in[:, :])
            iotaM = cpool.tile([P, P], fp)
            nc.sync.dma_start(iotaM[:], iota_in[:, :])
            q_sb = cpool.tile([P, T * F1], fp)
            u_sb = cpool.tile([P, T * F1], fp)

            # ---- phase A: T1 = dis * (x @ W1), spread into 256B tokens ----
            for t in range(T):
                rows = min(P, S - t * P)
                xt = xp.tile([P, D], fp, tag="xt")
                nc.sync.dma_start(xt[:rows, :], x_in[t * P:t * P + rows, :])
                hpt = hp.tile([P, F1], fp, space="PSUM", tag="hp")
                for k in range(D // P):
                    tp_ = pst.tile([P, P], fp, space="PSUM", tag="tp")
                    nc.tensor.transpose(
                        tp_[:, :rows], xt[:rows, k * P:(k + 1) * P],
                        ident[:rows, :rows],
                    )
                    xts = xtp.tile([P, P], fp, tag="xts")
                    nc.vector.tensor_copy(xts[:, :rows], tp_[:, :rows])
                    nc.tensor.matmul(
                        hpt[:rows, :], lhsT=xts[:, :rows],
                        rhs=w1s[:, k * F1:(k + 1) * F1],
                        start=(k == 0), stop=(k == D // P - 1),
                    )
                qsl = q_sb[:, t * F1:(t + 1) * F1]
                nc.vector.tensor_scalar(
                    qsl, hpt[:], disq[:, t:t + 1], None,
                    op0=mybir.AluOpType.mult,
                )
                nc.sync.dma_start(q2[t * P:t * P + rows, 0:F1], qsl[:rows, :])

            nc.gpsimd.collective_compute(
                "AllGather", mybir.AluOpType.bypass, replica_groups=groups,
                ins=[q2[:, :]], outs=[TAB1[0:N_NODES, :]],
            )

            def emit_pass(tab, own_sb, epi):
                col = 0
                wof = 0
                psblk = {}
                for q, wins in sched:
                    nw = len(wins)
                    it = sl.tile([P, WCALL * 8], mybir.dt.int16, tag="it")
                    nc.sync.dma_start(it[:, :nw * 8],
                                      idx_in[:, col:col + nw * 8])
                    dlt = dlp.tile([P, WCALL], fp, tag="dl")
                    nc.sync.dma_start(dlt[:, :nw], dl_in[:, wof:wof + nw])
                    G = gp.tile([P, WCALL * TOK], fp, tag="G")
                    Gv = G[:].rearrange("p (c e) -> p c e", e=TOK)
                    nc.gpsimd.dma_gather(
                        Gv[:, :nw, :], tab[q * QUAD:(q + 1) * QUAD, :],
                        it[:, :nw * 8], nw * P, regs[nw], TOK,
                    )
                    for j, (t, first, last) in enumerate(wins):
                        Dt = dp.tile([P, P], fp, tag="D")
                        nc.vector.tensor_scalar(
                            Dt[:], iotaM[:], dlt[:, j:j + 1], None,
                            op0=mybir.AluOpType.is_equal,
                        )
                        if first:
                            psblk[t] = aggp.tile(
                                [P, F1], fp, space="PSUM",
                                tag="agg", name=f"aggt{t}")
                        nc.tensor.matmul(
                            psblk[t][:], lhsT=Dt[:], rhs=Gv[:, j, 0:F1],
                            start=first, stop=last,
                        )
                        if last:
                            epi(t, psblk.pop(t)[:], own_sb)
                    col += nw * 8
                    wof += nw

            def epi1(t, pst_, own_sb):
                rows = min(P, S - t * P)
                a = ep.tile([P, F1], fp, tag="a")
                nc.vector.tensor_add(a[:], pst_,
                                     own_sb[:, t * F1:(t + 1) * F1])
                nc.vector.tensor_scalar(
                    a[:], a[:], disq[:, t:t + 1], None,
                    op0=mybir.AluOpType.mult,
                )
                nc.vector.tensor_add(a[:], a[:], b1t[:])
                usl = u_sb[:, t * F1:(t + 1) * F1]
                nc.vector.tensor_scalar(
                    usl, a[:], 0.0, disq[:, t:t + 1],
                    op0=mybir.AluOpType.max, op1=mybir.AluOpType.mult,
                )
                nc.sync.dma_start(u2[t * P:t * P + rows, 0:F1], usl[:rows, :])

            emit_pass(TAB1, q_sb, epi1)

            nc.gpsimd.collective_compute(
                "AllGather", mybir.AluOpType.bypass, replica_groups=groups,
                ins=[u2[:, :]], outs=[TAB2[0:N_NODES, :]],
            )

            def epi2(t, pst_, own_sb):
                rows = min(P, S - t * P)
                a = ep.tile([P, F1], fp, tag="a")
                nc.vector.tensor_add(a[:], pst_,
                                     own_sb[:, t * F1:(t + 1) * F1])
                v = ep.tile([P, F1], fp, tag="v")
                nc.vector.tensor_scalar(
                    v[:], a[:], disq[:, t:t + 1], None,
                    op0=mybir.AluOpType.mult,
                )
                vtp = pst.tile([P, P], fp, space="PSUM", tag="tp")
                nc.tensor.transpose(vtp[:F1, :], v[:, :], ident[:])
                vts = ep.tile([F1, P], fp, tag="vts")
                nc.vector.tensor_copy(vts[:, :], vtp[:F1, :])
                wp = wps.tile([P, F2], fp, space="PSUM", tag="wp")
                nc.tensor.matmul(wp[:], lhsT=vts[:, :], rhs=w2s[:, :],
                                 start=True, stop=True)
                w = ou.tile([P, F2], fp, tag="w")
                nc.vector.tensor_add(w[:], wp[:], b2t[:])
                mx = ep.tile([P, 1], fp, tag="mx")
                nc.vector.tensor_reduce(
                    out=mx[:], in_=w[:], op=mybir.AluOpType.max,
                    axis=mybir.AxisListType.X,
                )
                nmx = ep.tile([P, 1], fp, tag="nmx")
                nc.vector.tensor_scalar_mul(nmx[:], mx[:], -1.0)
                exv = ou.tile([P, F2], fp, tag="ex")
                se = ep.tile([P, 1], fp, tag="se")
                nc.scalar.activation(
                    exv[:], w[:], mybir.ActivationFunctionType.Exp,
                    bias=nmx[:], accum_out=se[:],
                )
                ls = ep.tile([P, 1], fp, tag="ls")
                nc.scalar.activation(ls[:], se[:],
                                     mybir.ActivationFunctionType.Ln)
                yt = ou.tile([P, F2], fp, tag="yt")
                nc.vector.tensor_scalar(
                    yt[:], w[:], mx[:], ls[:],
                    op0=mybir.AluOpType.subtract,
                    op1=mybir.AluOpType.subtract,
                )
                nc.sync.dma_start(y_out[t * P:t * P + rows, :], yt[:rows, :])

            emit_pass(TAB2, u_sb, epi2)

    lower_extended_insts(nc)
    _split_multi_waits(nc)
    return nc


# ---------------------------------------------------------------------------
class _Runner:
    def __init__(self, nc, n_cores):
        import jax
        from jax.sharding import Mesh, PartitionSpec
        from jax.experimental.shard_map import shard_map
        from concourse.bass2jax import (
            _bass_exec_p, partition_id_tensor, install_neuronx_cc_hook,
        )

        install_neuronx_cc_hook()
        self.jax = jax
        self.n_cores = n_cores
        in_names, out_names, out_avals = [], [], []
        partition_name = (
            nc.partition_id_tensor.name if nc.partition_id_tensor else None
        )
        for alloc in nc.m.functions[0].allocations:
            if not isinstance(alloc, mybir.MemoryLocationSet):
                continue
            name = alloc.memorylocations[0].name
            if alloc.kind == "ExternalInput":
                if name != partition_name:
                    in_names.append(name)
            elif alloc.kind == "ExternalOutput":
                out_names.append(name)
                out_avals.append(
                    jax.core.ShapedArray(
                        tuple(alloc.tensor_shape), mybir.dt.np(alloc.dtype)
                    )
                )
        self.in_names, self.out_names, self.out_avals = in_names, out_names, out_avals
        n_params, n_outs = len(in_names), len(out_avals)
        all_in = in_names + out_names
        if partition_name is not None:
            all_in.append(partition_name)

        def _body(*args):
            operands = list(args)
            if partition_name is not None:
                operands.append(partition_id_tensor())
            return tuple(
                _bass_exec_p.bind(
                    *operands, out_avals=tuple(out_avals), in_names=tuple(all_in),
                    out_names=tuple(out_names), lowering_input_output_aliases=(),
                    sim_require_finite=False, sim_require_nnan=False, nc=nc,
                )
            )

        devices = jax.devices()[:n_cores]
        mesh = Mesh(np.asarray(devices), ("core",))
        self.fn = jax.jit(
            shard_map(
                _body, mesh=mesh,
                in_specs=(PartitionSpec("core"),) * (n_params + n_outs),
                out_specs=(PartitionSpec("core"),) * n_outs,
                check_rep=False,
            ),
            keep_unused=True,
        )

    def run(self, in_maps):
        concat = [
            np.concatenate([np.asarray(m[name]) for m in in_maps], axis=0)
            for name in self.in_names
        ]
        zeros = [
            np.zeros((self.n_cores * a.shape[0], *a.shape[1:]), a.dtype)
            for a in self.out_avals
        ]
        out = self.fn(*concat, *zeros)
        self.jax.block_until_ready(out)
        res = []
        for c in range(self.n_cores):
            res.append({
                name: np.asarray(out[i]).reshape(
                    self.n_cores, *self.out_avals[i].shape
                )[c]
                for i, name in enumerate(self.out_names)
            })
        return res


_CACHE = {}


def _make_in_maps(x, W1, b1, W2, b2, percore):
    in_maps = []
    for c in range(N_CORES):
        in_maps.append({
            "x": np.asarray(x[c * S:(c + 1) * S], np.float32),
            "W1": np.asarray(W1, np.float32),
            "b1": np.asarray(b1, np.float32)[None],
            "W2": np.asarray(W2, np.float32),
            "b2": np.asarray(b2, np.float32)[None],
            "disq": percore["disq"][c],
            "iotaM": np.broadcast_to(
                np.arange(P, dtype=np.float32), (P, P)).copy(),
            "idx": percore["idx"][c],
            "dl": percore["dl"][c],
        })
    return in_maps


def kernel(x, edge_index, W1, b1, W2, b2):
    meta, percore = _preprocess(edge_index)
    key = ("gcn2", meta["idxcols"], meta["wtot"],
           tuple(len(w) for _, w in meta["sched"]))
    if key not in _CACHE:
        nc = _build_program(meta)
        _CACHE[key] = _Runner(nc, N_CORES)
    runner = _CACHE[key]

    res = runner.run(_make_in_maps(x, W1, b1, W2, b2, percore))
    y = np.empty((N_NODES, F2), np.float32)
    for c in range(N_CORES):
        y[c * S:(c + 1) * S] = res[c]["y"][:S]
    return y
